# revision 7
# baseline (speedup 1.0000x reference)
"""CLDice loss Trainium2 kernel.

Sharding: 8 cores = (batch b, z-half, y-half) quarters. Each core computes the
soft-skeletonization of its pred quarter (bf16, SBUF-resident, z on partitions)
and of its gt quarter (bit-packed int32 boolean morphology), with 12-voxel
redundant halos on the interior z/y cut sides; lower/right shards are
z/y-flipped by the host so the global volume border is always at index 0.
Per-core partial sums are combined on the host into the scalar loss.
"""
import numpy as np

NCORES = 8
Z = Y = X = 192
ZO = YO = 96          # owned extent per quarter
HALO = 12             # 11 erodes + 1 dilate
ZL = YL = ZO + HALO   # local ext extent (108)
NW = 8                # words per row in packed gt (cols 1..6 data, 0/7 pads)
ND = 6                # data words per row
YB = 12               # y-band rows for the float path
NIT = 11              # skeletonize iterations (incl. k=0 init)
FCH = 24              # final-pass row chunk

_CACHE = {}


def _build():
    import concourse.bacc as bacc
    import concourse.mybir as mybir
    from concourse import tile
    from concourse.alu_op_type import AluOpType as aop

    dt = mybir.dt
    AF = mybir.ActivationFunctionType
    nc = bacc.Bacc("TRN2", target_bir_lowering=False, debug=False,
                   num_devices=NCORES)

    lg_d = nc.dram_tensor("lg", [2, ZL, YL, X], dt.float32, kind="ExternalInput").ap()
    gtb_d = nc.dram_tensor("gtb", [ZL, YL, NW], dt.uint32, kind="ExternalInput").ap()
    p0_d = nc.dram_tensor("p0", [ZO, YO * X], dt.bfloat16).ap()
    sums_d = nc.dram_tensor("sums", [128, 16], dt.float32, kind="ExternalOutput").ap()

    ONES = 0xFFFFFFFF

    def bands(lo, hi, step):
        return [(y0, min(y0 + step, hi)) for y0 in range(lo, hi, step)]

    with tile.TileContext(nc) as tc:
        with tc.tile_pool(name="perm", bufs=1) as perm:
            skel = perm.tile([ZO, YO * X], dt.bfloat16)
            skg = perm.tile([ZO, YO * NW], dt.uint32)
            acc = perm.tile([128, 16], dt.float32)
            nc.vector.memset(acc[:, :], 0.0)

            with tc.tile_pool(name="epool", bufs=1) as ep:
                Ea = ep.tile([ZL, YL * X], dt.bfloat16, name="Ea")
                Eb = ep.tile([ZL, YL * X], dt.bfloat16, name="Eb")
                Ga = ep.tile([ZL, YL * NW], dt.uint32, name="Ga")
                Gb = ep.tile([ZL, YL * NW], dt.uint32, name="Gb")

                # ---------------- init: sigmoid(l1-l0) -> Ea, load gt bits ----
                with tc.tile_pool(name="initp", bufs=2) as ip:
                    nc.sync.dma_start(Ga[:, :], gtb_d[:, :, :])
                    for (y0, y1) in bands(0, YL, YB):
                        rr = y1 - y0
                        c0 = ip.tile([ZL, YB * X], dt.float32, tag="ic0")
                        c1 = ip.tile([ZL, YB * X], dt.float32, tag="ic1")
                        df = ip.tile([ZL, YB * X], dt.float32, tag="idf")
                        nc.sync.dma_start(c0[:, :rr * X], lg_d[0, :, y0:y1, :])
                        nc.sync.dma_start(c1[:, :rr * X], lg_d[1, :, y0:y1, :])
                        nc.vector.tensor_sub(df[:, :rr * X], c1[:, :rr * X], c0[:, :rr * X])
                        nc.scalar.activation(Ea[:, y0 * X:y1 * X], df[:, :rr * X], AF.Sigmoid)
                    nc.sync.dma_start(p0_d[:, :], Ea[0:ZO, 0:YO * X])

                # ---------------- helper: one float 3-tap pool band ----------
                def pool_band(src, dsttile, dst_is_view, y0, y1, op, bp):
                    """3x3x3 min/max pool of `src` rows [w0,w1) -> rows [y0,r_end)
                    written to dsttile (full [ZL,YL*X] view if dst_is_view else
                    band tile rows local 0..)."""
                    w0, w1 = max(y0 - 1, 0), min(y1 + 1, YL)
                    L = w1 - w0
                    r_end = min(y1, w1 - 1)
                    up = bp.tile([ZL, (YB + 2) * X], dt.bfloat16, tag="up", bufs=2)
                    dn = bp.tile([ZL, (YB + 2) * X], dt.bfloat16, tag="dn", bufs=2)
                    nc.sync.dma_start(up[0:ZL - 1, :L * X], src[1:ZL, w0 * X:w1 * X])
                    nc.sync.dma_start(up[ZL - 1:ZL, :L * X], src[ZL - 1:ZL, w0 * X:w1 * X])
                    nc.sync.dma_start(dn[1:ZL, :L * X], src[0:ZL - 1, w0 * X:w1 * X])
                    nc.sync.dma_start(dn[0:1, :L * X], src[0:1, w0 * X:w1 * X])
                    t1 = bp.tile([ZL, (YB + 2) * X], dt.bfloat16, tag="t1")
                    t2 = bp.tile([ZL, (YB + 2) * X], dt.bfloat16, tag="t2")
                    nc.vector.tensor_tensor(t1[:, :L * X], src[:, w0 * X:w1 * X], up[:, :L * X], op)
                    nc.vector.tensor_tensor(t2[:, :L * X], t1[:, :L * X], dn[:, :L * X], op)
                    # y stage: a_y[j] = op(t2[j], t2[j+1]) j in 0..L-2
                    t3 = bp.tile([ZL, (YB + 2) * X], dt.bfloat16, tag="t1")
                    nc.vector.tensor_tensor(t3[:, :(L - 1) * X], t2[:, 0:(L - 1) * X], t2[:, X:L * X], op)
                    # out rows [y0, r_end): out[r] = op(a_y[r-1-w0], a_y[r-w0]); r==0 -> a_y[0]
                    t4 = bp.tile([ZL, (YB + 2) * X], dt.bfloat16, tag="t2")
                    nr = 0
                    if y0 == 0:
                        nc.vector.tensor_copy(t4[:, 0:X], t3[:, 0:X])
                        if r_end > 1:
                            nc.vector.tensor_tensor(
                                t4[:, X:r_end * X], t3[:, 0:(r_end - 1) * X], t3[:, X:r_end * X], op)
                        nr = r_end
                    else:
                        nr = r_end - y0
                        j0 = y0 - w0  # == 1
                        nc.vector.tensor_tensor(
                            t4[:, 0:nr * X],
                            t3[:, (j0 - 1) * X:(j0 - 1 + nr) * X],
                            t3[:, j0 * X:(j0 + nr) * X], op)
                    # x stage on t4 rows 0..nr
                    t43 = t4.rearrange("p (r c) -> p r c", c=X)
                    t5 = bp.tile([ZL, (YB + 2) * X], dt.bfloat16, tag="t1")
                    t53 = t5.rearrange("p (r c) -> p r c", c=X)
                    nc.vector.tensor_tensor(t53[:, 0:nr, 0:X - 1], t43[:, 0:nr, 0:X - 1], t43[:, 0:nr, 1:X], op)
                    nc.vector.tensor_copy(t53[:, 0:nr, X - 1:X], t43[:, 0:nr, X - 1:X])
                    if dst_is_view:
                        d3 = dsttile.rearrange("p (r c) -> p r c", c=X)
                        o0, o1 = y0, y0 + nr
                    else:
                        d3 = dsttile.rearrange("p (r c) -> p r c", c=X)
                        o0, o1 = 0, nr
                    nc.vector.tensor_tensor(d3[:, o0:o1, 1:X], t53[:, 0:nr, 0:X - 1], t53[:, 0:nr, 1:X], op)
                    nc.vector.tensor_copy(d3[:, o0:o1, 0:1], t53[:, 0:nr, 0:1])
                    if dst_is_view and r_end < y1:
                        # fill clipped halo-edge rows with bounded values
                        nc.vector.tensor_copy(d3[:, r_end:y1, :], t53[:, nr - 1:nr, :])
                    return nr

                # ---------------- helper: packed-gt 3-tap pool ----------------
                def gt_pool(src, dst, op_and, bp):
                    """3x3x3 AND(erode)/OR(dilate) of packed src -> dst."""
                    top = aop.bitwise_and if op_and else aop.bitwise_or
                    # pads on src: ones for erode, zeros for dilate
                    s3 = src.rearrange("p (r w) -> p r w", w=NW)
                    nc.vector.memset(s3[:, :, 0:1], ONES if op_and else 0)
                    nc.vector.memset(s3[:, :, 7:8], ONES if op_and else 0)
                    FW = YL * NW
                    gu = bp.tile([ZL, FW], dt.uint32, tag="gu")
                    gd = bp.tile([ZL, FW], dt.uint32, tag="gd")
                    nc.sync.dma_start(gu[0:ZL - 1, :], src[1:ZL, :])
                    nc.sync.dma_start(gu[ZL - 1:ZL, :], src[ZL - 1:ZL, :])
                    nc.sync.dma_start(gd[1:ZL, :], src[0:ZL - 1, :])
                    nc.sync.dma_start(gd[0:1, :], src[0:1, :])
                    g1 = bp.tile([ZL, FW], dt.uint32, tag="g1")
                    g2 = bp.tile([ZL, FW], dt.uint32, tag="g2")
                    nc.vector.tensor_tensor(g1[:, :], src[:, :], gu[:, :], top)
                    nc.vector.tensor_tensor(g2[:, :], g1[:, :], gd[:, :], top)
                    # y stage
                    g13 = g1.rearrange("p (r w) -> p r w", w=NW)
                    g23 = g2.rearrange("p (r w) -> p r w", w=NW)
                    nc.vector.tensor_tensor(g13[:, 0:YL - 1, :], g23[:, 0:YL - 1, :], g23[:, 1:YL, :], top)
                    g3 = bp.tile([ZL, FW], dt.uint32, tag="g3")
                    g33 = g3.rearrange("p (r w) -> p r w", w=NW)
                    nc.vector.tensor_copy(g33[:, 0:1, :], g13[:, 0:1, :])
                    nc.vector.tensor_tensor(g33[:, 1:YL - 1, :], g13[:, 0:YL - 2, :], g13[:, 1:YL - 1, :], top)
                    nc.vector.tensor_copy(g33[:, YL - 1:YL, :], g13[:, YL - 2:YL - 1, :])
                    # x stage (bits, little endian: value(x+1) of bit b is bit b+1)
                    d3 = dst.rearrange("p (r w) -> p r w", w=NW)
                    s1 = bp.tile([ZL, FW], dt.uint32, tag="g2")
                    s13 = s1.rearrange("p (r w) -> p r w", w=NW)
                    s2 = bp.tile([ZL, FW], dt.uint32, tag="gu")
                    s23 = s2.rearrange("p (r w) -> p r w", w=NW)
                    # t_minus = (w<<1)|(w_prev>>31) ; t_plus = (w>>1)|(w_next<<31)
                    nc.vector.tensor_single_scalar(s13[:, :, 1:7], g33[:, :, 1:7], 1, aop.logical_shift_left)
                    nc.vector.tensor_single_scalar(s23[:, :, 1:7], g33[:, :, 0:6], 31, aop.logical_shift_right)
                    nc.vector.tensor_tensor(s13[:, :, 1:7], s13[:, :, 1:7], s23[:, :, 1:7], aop.bitwise_or)
                    nc.vector.tensor_tensor(s13[:, :, 1:7], s13[:, :, 1:7], g33[:, :, 1:7], top)
                    nc.vector.tensor_single_scalar(s23[:, :, 1:7], g33[:, :, 1:7], 1, aop.logical_shift_right)
                    nc.vector.tensor_tensor(d3[:, :, 1:7], s13[:, :, 1:7], s23[:, :, 1:7], top)
                    nc.vector.tensor_single_scalar(s23[:, :, 1:7], g33[:, :, 2:8], 31, aop.logical_shift_left)
                    nc.vector.tensor_tensor(d3[:, :, 1:7], d3[:, :, 1:7], s23[:, :, 1:7], top)

                # ---------------- main iterations -----------------------------
                with tc.tile_pool(name="bandp", bufs=1) as bp:
                    A, B = Ea, Eb
                    GA, GB = Ga, Gb
                    for k in range(NIT):
                        # erode sweep A -> B
                        for (y0, y1) in bands(0, YL, YB):
                            pool_band(A, B, True, y0, y1, aop.min, bp)
                        # gt erode GA -> GB
                        gt_pool(GA, GB, True, bp)
                        # dilate bands of B + delta/skel on owned rows
                        for (y0, y1) in bands(0, YO, YB):
                            D = bp.tile([ZL, (YB + 2) * X], dt.bfloat16, tag="dd")
                            nr = pool_band(B, D, False, y0, y1, aop.max, bp)
                            rr = min(y1, YO) - y0
                            sub = bp.tile([ZO, YB * X], dt.bfloat16, tag="sub")
                            nc.vector.tensor_sub(
                                sub[:, :rr * X], A[0:ZO, y0 * X:(y0 + rr) * X], D[0:ZO, 0:rr * X])
                            if k == 0:
                                nc.scalar.activation(
                                    skel[:, y0 * X:(y0 + rr) * X], sub[:, :rr * X], AF.Relu)
                            else:
                                tne = bp.tile([ZO, YB * X], dt.bfloat16, tag="tne")
                                nc.scalar.activation(
                                    tne[:, :rr * X], skel[:, y0 * X:(y0 + rr) * X],
                                    AF.Copy, scale=-1.0, bias=1.0)
                                nc.vector.tensor_mul(sub[:, :rr * X], sub[:, :rr * X], tne[:, :rr * X])
                                nc.scalar.activation(tne[:, :rr * X], sub[:, :rr * X], AF.Relu)
                                nc.vector.tensor_add(
                                    skel[:, y0 * X:(y0 + rr) * X],
                                    skel[:, y0 * X:(y0 + rr) * X], tne[:, :rr * X])
                        # gt dilate + delta/skel_gt
                        GD = bp.tile([ZL, YL * NW], dt.uint32, tag="gdl")
                        gt_pool(GB, GD, False, bp)
                        gnt = bp.tile([ZL, YL * NW], dt.uint32, tag="g1")
                        gnt3 = gnt.rearrange("p (r w) -> p r w", w=NW)
                        GD3 = GD.rearrange("p (r w) -> p r w", w=NW)
                        GA3 = GA.rearrange("p (r w) -> p r w", w=NW)
                        skg3 = skg.rearrange("p (r w) -> p r w", w=NW)
                        nc.vector.tensor_single_scalar(gnt3[:, :, 1:7], GD3[:, :, 1:7], ONES, aop.bitwise_xor)
                        nc.vector.tensor_tensor(gnt3[:, :, 1:7], GA3[:, :, 1:7], gnt3[:, :, 1:7], aop.bitwise_and)
                        if k == 0:
                            nc.vector.tensor_copy(skg3[:, :, 1:7], gnt3[0:ZO, 0:YO, 1:7])
                        else:
                            nc.vector.tensor_tensor(
                                skg3[:, :, 1:7], skg3[:, :, 1:7], gnt3[0:ZO, 0:YO, 1:7], aop.bitwise_or)
                        A, B = B, A
                        GA, GB = GB, GA

            # ---------------- final: partial sums --------------------------
            with tc.tile_pool(name="finp", bufs=1) as fp:
                for ci, (y0, y1) in enumerate(bands(0, YO, FCH)):
                    rr = y1 - y0
                    FR = rr * X
                    gtw = fp.tile([ZO, FCH * NW], dt.uint32, tag="fgw")
                    nc.sync.dma_start(gtw[:, :rr * NW], gtb_d[0:ZO, y0:y1, :])
                    mi = fp.tile([ZO, FCH * X], dt.uint32, tag="fmi")
                    mi4 = mi.rearrange("p (r w b) -> p r w b", w=ND, b=32)
                    gw4 = gtw.rearrange("p (r w) -> p r w", w=NW)
                    for b in range(32):
                        nc.vector.tensor_scalar(
                            mi4[:, 0:rr, :, b], gw4[:, 0:rr, 1:7], b, 1,
                            aop.logical_shift_right, aop.bitwise_and)
                    mb = fp.tile([ZO, FCH * X], dt.bfloat16, tag="fmb")
                    nc.vector.tensor_copy(mb[:, :FR], mi[:, :FR])
                    scr = fp.tile([ZO, FCH * X], dt.bfloat16, tag="fsc")
                    # S1 = sum(skel_pred * gt)
                    nc.vector.scalar_tensor_tensor(
                        scr[:, :FR], skel[:, y0 * X:y1 * X], 1.0, mb[:, :FR],
                        aop.mult, aop.mult, accum_out=acc[0:ZO, ci:ci + 1])
                    # S2 = sum(skel_pred)
                    nc.vector.tensor_scalar(
                        scr[:, :FR], skel[:, y0 * X:y1 * X], 0.0, 0.0,
                        aop.add, aop.add, accum_out=acc[0:ZO, 4 + ci:5 + ci])
                    # unpack skel_gt
                    sg4 = skg.rearrange("p (r w) -> p r w", w=NW)
                    for b in range(32):
                        nc.vector.tensor_scalar(
                            mi4[:, 0:rr, :, b], sg4[:, y0:y1, 1:7], b, 1,
                            aop.logical_shift_right, aop.bitwise_and)
                    nc.vector.tensor_copy(mb[:, :FR], mi[:, :FR])
                    # S4 = sum(skel_gt)
                    nc.vector.tensor_scalar(
                        scr[:, :FR], mb[:, :FR], 0.0, 0.0,
                        aop.add, aop.add, accum_out=acc[0:ZO, 12 + ci:13 + ci])
                    # S3 = sum(skel_gt * pred)
                    pt = fp.tile([ZO, FCH * X], dt.bfloat16, tag="fpt")
                    nc.sync.dma_start(pt[:, :FR], p0_d[:, y0 * X:y1 * X])
                    nc.vector.scalar_tensor_tensor(
                        scr[:, :FR], mb[:, :FR], 1.0, pt[:, :FR],
                        aop.mult, aop.mult, accum_out=acc[0:ZO, 8 + ci:9 + ci])
                nc.sync.dma_start(sums_d[:, :], acc[:, :])

    nc.compile()
    return nc


def _host_shard(logits, targets):
    logits = np.ascontiguousarray(np.asarray(logits, dtype=np.float32))
    targets = np.asarray(targets)
    in_maps = []
    for c in range(NCORES):
        b, zh, yh = c >> 2, (c >> 1) & 1, c & 1
        lg = logits[b]
        gt = (targets[b] == 1)
        if zh:
            lg = lg[:, ::-1]
            gt = gt[::-1]
        if yh:
            lg = lg[:, :, ::-1]
            gt = gt[:, ::-1]
        lg = np.ascontiguousarray(lg[:, :ZL, :YL])            # (2, ZL, YL, X)
        gt = np.ascontiguousarray(gt[:ZL, :YL])               # (ZL, YL, X) bool
        words = np.packbits(gt, axis=-1, bitorder="little")   # (ZL, YL, 24) u8
        words = words.view(np.uint32)                         # (ZL, YL, 6)
        gtb = np.zeros((ZL, YL, NW), dtype=np.uint32)
        gtb[:, :, 1:7] = words
        in_maps.append({"lg": lg, "gtb": gtb})
    return in_maps


def kernel(logits, targets):
    from concourse.bass_utils import run_bass_kernel_spmd
    if "nc" not in _CACHE:
        _CACHE["nc"] = _build()
    nc = _CACHE["nc"]
    in_maps = _host_shard(logits, targets)
    res = run_bass_kernel_spmd(nc, in_maps, list(range(NCORES)), trace=False)
    S = np.zeros(4, dtype=np.float64)
    for r in res.results:
        a = r["sums"].astype(np.float64)
        S[0] += a[:, 0:4].sum()
        S[1] += a[:, 4:8].sum()
        S[2] += a[:, 8:12].sum()
        S[3] += a[:, 12:16].sum()
    tprec = (S[0] + 1.0) / (S[1] + 1.0)
    tsens = (S[2] + 1.0) / (S[3] + 1.0)
    cl = 2.0 * tprec * tsens / (tprec + tsens + 1e-7)
    return np.float32(1.0 - cl)


# revision 12
# speedup vs baseline: 1.0027x; 1.0027x over previous
"""CLDice loss Trainium2 kernel.

Sharding: 8 cores = (batch b, z-half, y-half) quarters. Each core computes the
soft-skeletonization of its pred quarter (bf16, SBUF-resident, z on partitions)
and of its gt quarter (bit-packed int32 boolean morphology), with 12-voxel
redundant halos on the interior z/y cut sides; lower/right shards are
z/y-flipped by the host so the global volume border is always at index 0.
Per-core partial sums are combined on the host into the scalar loss.
"""
import numpy as np

NCORES = 8
Z = Y = X = 192
ZO = YO = 96          # owned extent per quarter
HALO = 12             # 11 erodes + 1 dilate
ZL = YL = ZO + HALO   # local ext extent (108)
NW = 8                # words per row in packed gt (cols 1..6 data, 0/7 pads)
ND = 6                # data words per row
YB = 12               # y-band rows for the float path
NIT = 11              # skeletonize iterations (incl. k=0 init)
FCH = 24              # final-pass row chunk

_CACHE = {}


def _build():
    import concourse.bacc as bacc
    import concourse.mybir as mybir
    from concourse import tile
    from concourse.alu_op_type import AluOpType as aop

    dt = mybir.dt
    AF = mybir.ActivationFunctionType
    nc = bacc.Bacc("TRN2", target_bir_lowering=False, debug=False,
                   num_devices=NCORES)

    lg_d = nc.dram_tensor("lg", [2, ZL, YL, X], dt.float32, kind="ExternalInput").ap()
    gtb_d = nc.dram_tensor("gtb", [ZL, YL, NW], dt.uint32, kind="ExternalInput").ap()
    p0_d = nc.dram_tensor("p0", [ZO, YO * X], dt.bfloat16).ap()
    sums_d = nc.dram_tensor("sums", [128, 16], dt.float32, kind="ExternalOutput").ap()

    ONES = 0xFFFFFFFF

    def bands(lo, hi, step):
        return [(y0, min(y0 + step, hi)) for y0 in range(lo, hi, step)]

    with tile.TileContext(nc) as tc:
        with tc.tile_pool(name="perm", bufs=1) as perm:
            skel = perm.tile([ZO, YO * X], dt.bfloat16)
            skg = perm.tile([ZO, YO * NW], dt.uint32)
            acc = perm.tile([128, 16], dt.float32)
            nc.vector.memset(acc[:, :], 0.0)

            with tc.tile_pool(name="epool", bufs=1) as ep:
                Ea = ep.tile([ZL, YL * X], dt.bfloat16, name="Ea")
                Eb = ep.tile([ZL, YL * X], dt.bfloat16, name="Eb")
                Ga = ep.tile([ZL, YL * NW], dt.uint32, name="Ga")
                Gb = ep.tile([ZL, YL * NW], dt.uint32, name="Gb")

                # ---------------- init: sigmoid(l1-l0) -> Ea, load gt bits ----
                with tc.tile_pool(name="initp", bufs=2) as ip:
                    nc.sync.dma_start(Ga[:, :], gtb_d[:, :, :])
                    for (y0, y1) in bands(0, YL, YB):
                        rr = y1 - y0
                        c0 = ip.tile([ZL, YB * X], dt.float32, tag="ic0")
                        c1 = ip.tile([ZL, YB * X], dt.float32, tag="ic1")
                        df = ip.tile([ZL, YB * X], dt.float32, tag="idf")
                        nc.sync.dma_start(c0[:, :rr * X], lg_d[0, :, y0:y1, :])
                        nc.sync.dma_start(c1[:, :rr * X], lg_d[1, :, y0:y1, :])
                        nc.vector.tensor_sub(df[:, :rr * X], c1[:, :rr * X], c0[:, :rr * X])
                        nc.scalar.activation(Ea[:, y0 * X:y1 * X], df[:, :rr * X], AF.Sigmoid)
                    nc.sync.dma_start(p0_d[:, :], Ea[0:ZO, 0:YO * X])

                # ---------------- helper: one float 3-tap pool band ----------
                def pool_band(src, dsttile, dst_is_view, y0, y1, op, bp, shift_tiles, bidx):
                    """3x3x3 min/max pool of `src` rows [w0,w1) -> rows [y0,r_end)
                    written to dsttile (full [ZL,YL*X] view if dst_is_view else
                    band tile rows local 0..)."""
                    w0, w1 = max(y0 - 1, 0), min(y1 + 1, YL)
                    L = w1 - w0
                    r_end = min(y1, w1 - 1)
                    up_t, dn_et, dn_dt = shift_tiles[0], shift_tiles[1], shift_tiles[2]
                    up = up_t[bidx % 2]
                    dn = (dn_et if op == aop.min else dn_dt)[bidx % 2]
                    nc.sync.dma_start(up[0:ZL - 1, :L * X], src[1:ZL, w0 * X:w1 * X])
                    nc.sync.dma_start(dn[1:ZL, :L * X], src[0:ZL - 1, w0 * X:w1 * X])
                    t1 = bp.tile([ZL, (YB + 2) * X], dt.bfloat16, tag="t1")
                    t2 = bp.tile([ZL, (YB + 2) * X], dt.bfloat16, tag="t2")
                    nc.vector.tensor_tensor(t1[:, :L * X], src[:, w0 * X:w1 * X], up[:, :L * X], op)
                    nc.vector.tensor_tensor(t2[:, :L * X], t1[:, :L * X], dn[:, :L * X], op)
                    # y stage: a_y[j] = op(t2[j], t2[j+1]) j in 0..L-2
                    t3 = bp.tile([ZL, (YB + 2) * X], dt.bfloat16, tag="t1")
                    nc.vector.tensor_tensor(t3[:, :(L - 1) * X], t2[:, 0:(L - 1) * X], t2[:, X:L * X], op)
                    # out rows [y0, r_end): out[r] = op(a_y[r-1-w0], a_y[r-w0]); r==0 -> a_y[0]
                    t4 = bp.tile([ZL, (YB + 2) * X], dt.bfloat16, tag="t2")
                    nr = 0
                    if y0 == 0:
                        nc.vector.tensor_copy(t4[:, 0:X], t3[:, 0:X])
                        if r_end > 1:
                            nc.vector.tensor_tensor(
                                t4[:, X:r_end * X], t3[:, 0:(r_end - 1) * X], t3[:, X:r_end * X], op)
                        nr = r_end
                    else:
                        nr = r_end - y0
                        j0 = y0 - w0  # == 1
                        nc.vector.tensor_tensor(
                            t4[:, 0:nr * X],
                            t3[:, (j0 - 1) * X:(j0 - 1 + nr) * X],
                            t3[:, j0 * X:(j0 + nr) * X], op)
                    # x stage on t4 rows 0..nr
                    t43 = t4.rearrange("p (r c) -> p r c", c=X)
                    t5 = bp.tile([ZL, (YB + 2) * X], dt.bfloat16, tag="t1")
                    t53 = t5.rearrange("p (r c) -> p r c", c=X)
                    nc.vector.tensor_tensor(t53[:, 0:nr, 0:X - 1], t43[:, 0:nr, 0:X - 1], t43[:, 0:nr, 1:X], op)
                    nc.vector.tensor_copy(t53[:, 0:nr, X - 1:X], t43[:, 0:nr, X - 1:X])
                    if dst_is_view:
                        d3 = dsttile.rearrange("p (r c) -> p r c", c=X)
                        o0, o1 = y0, y0 + nr
                    else:
                        d3 = dsttile.rearrange("p (r c) -> p r c", c=X)
                        o0, o1 = 0, nr
                    nc.vector.tensor_tensor(d3[:, o0:o1, 1:X], t53[:, 0:nr, 0:X - 1], t53[:, 0:nr, 1:X], op)
                    nc.vector.tensor_copy(d3[:, o0:o1, 0:1], t53[:, 0:nr, 0:1])
                    if dst_is_view and r_end < y1:
                        # fill clipped halo-edge rows with bounded values
                        nc.vector.tensor_copy(d3[:, r_end:y1, :], t53[:, nr - 1:nr, :])
                    return nr

                # ---------------- helper: packed-gt 3-tap pool ----------------
                def gt_pool(src, dst, op_and, bp, shift_tiles):
                    """3x3x3 AND(erode)/OR(dilate) of packed src -> dst."""
                    top = aop.bitwise_and if op_and else aop.bitwise_or
                    # pads on src: ones for erode, zeros for dilate
                    s3 = src.rearrange("p (r w) -> p r w", w=NW)
                    nc.vector.memset(s3[:, :, 0:1], ONES if op_and else 0)
                    nc.vector.memset(s3[:, :, 7:8], ONES if op_and else 0)
                    FW = YL * NW
                    gu = shift_tiles[3]
                    gd = shift_tiles[4] if op_and else shift_tiles[5]
                    nc.sync.dma_start(gu[0:ZL - 1, :], src[1:ZL, :])
                    nc.sync.dma_start(gd[1:ZL, :], src[0:ZL - 1, :])
                    g1 = bp.tile([ZL, FW], dt.uint32, tag="g1")
                    g2 = bp.tile([ZL, FW], dt.uint32, tag="g2")
                    nc.vector.tensor_tensor(g1[:, :], src[:, :], gu[:, :], top)
                    nc.vector.tensor_tensor(g2[:, :], g1[:, :], gd[:, :], top)
                    # y stage
                    g13 = g1.rearrange("p (r w) -> p r w", w=NW)
                    g23 = g2.rearrange("p (r w) -> p r w", w=NW)
                    nc.vector.tensor_tensor(g13[:, 0:YL - 1, :], g23[:, 0:YL - 1, :], g23[:, 1:YL, :], top)
                    g3 = bp.tile([ZL, FW], dt.uint32, tag="g3")
                    g33 = g3.rearrange("p (r w) -> p r w", w=NW)
                    nc.vector.tensor_copy(g33[:, 0:1, :], g13[:, 0:1, :])
                    nc.vector.tensor_tensor(g33[:, 1:YL - 1, :], g13[:, 0:YL - 2, :], g13[:, 1:YL - 1, :], top)
                    nc.vector.tensor_copy(g33[:, YL - 1:YL, :], g13[:, YL - 2:YL - 1, :])
                    # x stage (bits, little endian: value(x+1) of bit b is bit b+1)
                    d3 = dst.rearrange("p (r w) -> p r w", w=NW)
                    s1 = bp.tile([ZL, FW], dt.uint32, tag="g2")
                    s13 = s1.rearrange("p (r w) -> p r w", w=NW)
                    s2 = bp.tile([ZL, FW], dt.uint32, tag="gu")
                    s23 = s2.rearrange("p (r w) -> p r w", w=NW)
                    # t_minus = (w<<1)|(w_prev>>31) ; t_plus = (w>>1)|(w_next<<31)
                    nc.vector.tensor_single_scalar(s13[:, :, 1:7], g33[:, :, 1:7], 1, aop.logical_shift_left)
                    nc.vector.tensor_single_scalar(s23[:, :, 1:7], g33[:, :, 0:6], 31, aop.logical_shift_right)
                    nc.vector.tensor_tensor(s13[:, :, 1:7], s13[:, :, 1:7], s23[:, :, 1:7], aop.bitwise_or)
                    nc.vector.tensor_tensor(s13[:, :, 1:7], s13[:, :, 1:7], g33[:, :, 1:7], top)
                    nc.vector.tensor_single_scalar(s23[:, :, 1:7], g33[:, :, 1:7], 1, aop.logical_shift_right)
                    nc.vector.tensor_tensor(d3[:, :, 1:7], s13[:, :, 1:7], s23[:, :, 1:7], top)
                    nc.vector.tensor_single_scalar(s23[:, :, 1:7], g33[:, :, 2:8], 31, aop.logical_shift_left)
                    nc.vector.tensor_tensor(d3[:, :, 1:7], d3[:, :, 1:7], s23[:, :, 1:7], top)

                # ---------------- main iterations -----------------------------
                with tc.tile_pool(name="bandp", bufs=1) as bp:
                    # static shift-buffer tiles: edge rows written once here,
                    # never touched by the per-band DMAs
                    up_t, dn_et, dn_dt = [], [], []
                    for j in range(2):
                        t_ = bp.tile([ZL, (YB + 2) * X], dt.bfloat16, name=f"upt{j}")
                        nc.vector.memset(t_[:, :], 0.5)
                        up_t.append(t_)
                        t_ = bp.tile([ZL, (YB + 2) * X], dt.bfloat16, name=f"dnet{j}")
                        nc.vector.memset(t_[:, :], 1.0)
                        dn_et.append(t_)
                        t_ = bp.tile([ZL, (YB + 2) * X], dt.bfloat16, name=f"dndt{j}")
                        nc.vector.memset(t_[:, :], 0.0)
                        dn_dt.append(t_)
                    gu_t = bp.tile([ZL, YL * NW], dt.uint32, name="gut")
                    nc.vector.memset(gu_t[:, :], 0)
                    gd_et = bp.tile([ZL, YL * NW], dt.uint32, name="gdet")
                    nc.vector.memset(gd_et[:, :], ONES)
                    gd_dt = bp.tile([ZL, YL * NW], dt.uint32, name="gddt")
                    nc.vector.memset(gd_dt[:, :], 0)
                    shift_tiles = (up_t, dn_et, dn_dt, gu_t, gd_et, gd_dt)
                    A, B = Ea, Eb
                    GA, GB = Ga, Gb
                    for k in range(NIT):
                        # erode sweep A -> B
                        for bi, (y0, y1) in enumerate(bands(0, YL, YB)):
                            pool_band(A, B, True, y0, y1, aop.min, bp, shift_tiles, bi)
                        # gt erode GA -> GB
                        gt_pool(GA, GB, True, bp, shift_tiles)
                        # dilate bands of B + delta/skel on owned rows
                        for bi, (y0, y1) in enumerate(bands(0, YO, YB)):
                            D = bp.tile([ZL, (YB + 2) * X], dt.bfloat16, tag="dd")
                            nr = pool_band(B, D, False, y0, y1, aop.max, bp, shift_tiles, bi)
                            rr = min(y1, YO) - y0
                            sub = bp.tile([ZL, (YB + 2) * X], dt.bfloat16, tag="t2")
                            nc.vector.tensor_sub(
                                sub[0:ZO, :rr * X], A[0:ZO, y0 * X:(y0 + rr) * X], D[0:ZO, 0:rr * X])
                            if k == 0:
                                nc.scalar.activation(
                                    skel[:, y0 * X:(y0 + rr) * X], sub[0:ZO, :rr * X], AF.Relu)
                            else:
                                tne = bp.tile([ZL, (YB + 2) * X], dt.bfloat16, tag="t1")
                                nc.scalar.activation(
                                    tne[0:ZO, :rr * X], skel[:, y0 * X:(y0 + rr) * X],
                                    AF.Copy, scale=-1.0, bias=1.0)
                                nc.gpsimd.tensor_mul(sub[0:ZO, :rr * X], sub[0:ZO, :rr * X], tne[0:ZO, :rr * X])
                                nc.scalar.activation(tne[0:ZO, :rr * X], sub[0:ZO, :rr * X], AF.Relu)
                                nc.gpsimd.tensor_add(
                                    skel[:, y0 * X:(y0 + rr) * X],
                                    skel[:, y0 * X:(y0 + rr) * X], tne[0:ZO, :rr * X])
                        # gt dilate + delta/skel_gt
                        GD = bp.tile([ZL, YL * NW], dt.uint32, tag="gdl")
                        gt_pool(GB, GD, False, bp, shift_tiles)
                        gnt = bp.tile([ZL, YL * NW], dt.uint32, tag="g1")
                        gnt3 = gnt.rearrange("p (r w) -> p r w", w=NW)
                        GD3 = GD.rearrange("p (r w) -> p r w", w=NW)
                        GA3 = GA.rearrange("p (r w) -> p r w", w=NW)
                        skg3 = skg.rearrange("p (r w) -> p r w", w=NW)
                        nc.vector.tensor_single_scalar(gnt3[:, :, 1:7], GD3[:, :, 1:7], ONES, aop.bitwise_xor)
                        nc.vector.tensor_tensor(gnt3[:, :, 1:7], GA3[:, :, 1:7], gnt3[:, :, 1:7], aop.bitwise_and)
                        if k == 0:
                            nc.vector.tensor_copy(skg3[:, :, 1:7], gnt3[0:ZO, 0:YO, 1:7])
                        else:
                            nc.vector.tensor_tensor(
                                skg3[:, :, 1:7], skg3[:, :, 1:7], gnt3[0:ZO, 0:YO, 1:7], aop.bitwise_or)
                        A, B = B, A
                        GA, GB = GB, GA

            # ---------------- final: partial sums --------------------------
            with tc.tile_pool(name="finp", bufs=1) as fp:
                for ci, (y0, y1) in enumerate(bands(0, YO, FCH)):
                    rr = y1 - y0
                    FR = rr * X
                    gtw = fp.tile([ZO, FCH * NW], dt.uint32, tag="fgw")
                    nc.sync.dma_start(gtw[:, :rr * NW], gtb_d[0:ZO, y0:y1, :])
                    mi = fp.tile([ZO, FCH * X], dt.uint32, tag="fmi")
                    mi4 = mi.rearrange("p (r w b) -> p r w b", w=ND, b=32)
                    gw4 = gtw.rearrange("p (r w) -> p r w", w=NW)
                    for b in range(32):
                        nc.vector.tensor_scalar(
                            mi4[:, 0:rr, :, b], gw4[:, 0:rr, 1:7], b, 1,
                            aop.logical_shift_right, aop.bitwise_and)
                    mb = fp.tile([ZO, FCH * X], dt.bfloat16, tag="fmb")
                    nc.vector.tensor_copy(mb[:, :FR], mi[:, :FR])
                    scr = fp.tile([ZO, FCH * X], dt.bfloat16, tag="fsc")
                    # S1 = sum(skel_pred * gt)
                    nc.vector.scalar_tensor_tensor(
                        scr[:, :FR], skel[:, y0 * X:y1 * X], 1.0, mb[:, :FR],
                        aop.mult, aop.mult, accum_out=acc[0:ZO, ci:ci + 1])
                    # S2 = sum(skel_pred)
                    nc.vector.tensor_scalar(
                        scr[:, :FR], skel[:, y0 * X:y1 * X], 0.0, 0.0,
                        aop.add, aop.add, accum_out=acc[0:ZO, 4 + ci:5 + ci])
                    # unpack skel_gt
                    sg4 = skg.rearrange("p (r w) -> p r w", w=NW)
                    for b in range(32):
                        nc.vector.tensor_scalar(
                            mi4[:, 0:rr, :, b], sg4[:, y0:y1, 1:7], b, 1,
                            aop.logical_shift_right, aop.bitwise_and)
                    nc.vector.tensor_copy(mb[:, :FR], mi[:, :FR])
                    # S4 = sum(skel_gt)
                    nc.vector.tensor_scalar(
                        scr[:, :FR], mb[:, :FR], 0.0, 0.0,
                        aop.add, aop.add, accum_out=acc[0:ZO, 12 + ci:13 + ci])
                    # S3 = sum(skel_gt * pred)
                    pt = fp.tile([ZO, FCH * X], dt.bfloat16, tag="fpt")
                    nc.sync.dma_start(pt[:, :FR], p0_d[:, y0 * X:y1 * X])
                    nc.vector.scalar_tensor_tensor(
                        scr[:, :FR], mb[:, :FR], 1.0, pt[:, :FR],
                        aop.mult, aop.mult, accum_out=acc[0:ZO, 8 + ci:9 + ci])
                nc.sync.dma_start(sums_d[:, :], acc[:, :])

    nc.compile()
    return nc


def _host_shard(logits, targets):
    logits = np.ascontiguousarray(np.asarray(logits, dtype=np.float32))
    targets = np.asarray(targets)
    in_maps = []
    for c in range(NCORES):
        b, zh, yh = c >> 2, (c >> 1) & 1, c & 1
        lg = logits[b]
        gt = (targets[b] == 1)
        if zh:
            lg = lg[:, ::-1]
            gt = gt[::-1]
        if yh:
            lg = lg[:, :, ::-1]
            gt = gt[:, ::-1]
        lg = np.ascontiguousarray(lg[:, :ZL, :YL])            # (2, ZL, YL, X)
        gt = np.ascontiguousarray(gt[:ZL, :YL])               # (ZL, YL, X) bool
        words = np.packbits(gt, axis=-1, bitorder="little")   # (ZL, YL, 24) u8
        words = words.view(np.uint32)                         # (ZL, YL, 6)
        gtb = np.zeros((ZL, YL, NW), dtype=np.uint32)
        gtb[:, :, 1:7] = words
        in_maps.append({"lg": lg, "gtb": gtb})
    return in_maps


def kernel(logits, targets):
    from concourse.bass_utils import run_bass_kernel_spmd
    if "nc" not in _CACHE:
        _CACHE["nc"] = _build()
    nc = _CACHE["nc"]
    in_maps = _host_shard(logits, targets)
    res = run_bass_kernel_spmd(nc, in_maps, list(range(NCORES)), trace=False)
    S = np.zeros(4, dtype=np.float64)
    for r in res.results:
        a = r["sums"].astype(np.float64)
        S[0] += a[:, 0:4].sum()
        S[1] += a[:, 4:8].sum()
        S[2] += a[:, 8:12].sum()
        S[3] += a[:, 12:16].sum()
    tprec = (S[0] + 1.0) / (S[1] + 1.0)
    tsens = (S[2] + 1.0) / (S[3] + 1.0)
    cl = 2.0 * tprec * tsens / (tprec + tsens + 1e-7)
    return np.float32(1.0 - cl)
